# revision 4
# baseline (speedup 1.0000x reference)
"""MoE gate (router) kernel for Trainium2.

Computes, for hidden_states [T, H] and gate weight [E, H]:
    logits = hidden_states @ weight.T          # [T, E]
    probs  = softmax(logits, axis=-1)
    topk_weight, topk_idx = top_k(probs, 8)    # normalized over the top-8
    row_idx = arange(T*8).reshape(8, T).T

Strategy (8 NeuronCores, data parallel over tokens):
  - Split hs = hhi + hlo/2^11 and 64*w = whi + wlo/2^11 (fp16 hi parts).
    logits*64 = hhi.whi  +  (hhi.wlo + hlo.whi)/2^11  (+ ~2^-22 dropped).
  - Main term hhi.whi in fp16 (1 cycle/row).  Both cross terms run in
    fp8e4 with the DoubleRow perf mode (0.5 cycles/row): adjacent k-tiles
    are paired as the two DoubleRow planes, so 28 DR matmuls cover 56
    k-tiles of hi.wlo and another 28 cover lo.whi.  Per k-tile the PE does
    256 (fp16) + 128 (fp8 DR) column-cycles = 384, vs 768 for an all-fp16
    hi/lo scheme and 1024 for native fp32.  The fp8 quantization only
    perturbs terms at 2^-11 scale: measured ~15/131072 top-8 index flips
    (idx rel err 7.6e-3, gate is 2e-2).
  - hs ships as fp16 hi (2B) + fp8 lo (1B) = 3 B/elem; the fp8 copy of
    the hi part is cast on-device by the idle GpSimd/Pool engine, off the
    critical path.  All DMAs use per-partition-contiguous runs (>= 7KB,
    128 descriptors each) so the DMA engines run at full bus width.
  - Top-8 via DVE max/max_index on the raw 64x-scaled logits (order
    invariant); the 1/64 folds into the exp activation's scale.  Softmax
    over 256 + top-k renorm reduces to softmax over the top-8 logits.
"""

import numpy as np

TOP_K = 8
NUM_EXPERTS = 256
HIDDEN = 7168
NUM_TOKENS = 16384
N_CORES = 8
T_LOC = NUM_TOKENS // N_CORES

W_SCALE = 64.0       # weight pre-scale so fp16(64*w) stays normal-range
LO_SCALE = 2048.0    # 2^11: lo parts carry the next 11 mantissa bits

P = 128
KT = HIDDEN // P     # 56 k-tiles along hidden dim
KP = KT // 2         # 28 DoubleRow k-pairs
TS = T_LOC // P      # 16 128-token subtiles per core

_NC_CACHE = {}


def build_gate_nc(t_loc=T_LOC, h=HIDDEN, e=NUM_EXPERTS, repeat=1):
    import concourse.mybir as mybir
    import concourse.tile as tile
    from concourse import bacc

    f32 = mybir.dt.float32
    fp16 = mybir.dt.float16
    fp8 = mybir.dt.float8e4
    DR = mybir.MatmulPerfMode.DoubleRow
    kt = h // P
    kp = kt // 2
    ts_n = t_loc // P

    nc = bacc.Bacc("TRN2", target_bir_lowering=False)
    # hs_hi[p, ts, ko, t] fp16, hs_lo8[p, ts, ko, t] fp8: hidden = ko*128+p,
    # token = ts*128+t.  Per-partition contiguous runs of kt*128 elements.
    hs_hi = nc.dram_tensor("hs_hi", [P, ts_n * kt * P], fp16, kind="ExternalInput")
    hs_lo8 = nc.dram_tensor("hs_lo8", [P, ts_n * kt * P], fp8, kind="ExternalInput")
    # weights: whi fp16 + fp8 copies of whi and wlo (all [p, ko, e])
    w_hi = nc.dram_tensor("w_hi", [P, kt * e], fp16, kind="ExternalInput")
    w_hi8 = nc.dram_tensor("w_hi8", [P, kt * e], fp8, kind="ExternalInput")
    w_lo8 = nc.dram_tensor("w_lo8", [P, kt * e], fp8, kind="ExternalInput")
    idx_out = nc.dram_tensor(
        "topk_idx", [t_loc, TOP_K], mybir.dt.int32, kind="ExternalOutput"
    )
    w_out = nc.dram_tensor("topk_w", [t_loc, TOP_K], f32, kind="ExternalOutput")

    hhi_v = hs_hi[:, :].rearrange("p (ts ko t) -> p ts ko t", ts=ts_n, ko=kt)
    hlo8_v = hs_lo8[:, :].rearrange("p (ts ko t) -> p ts ko t", ts=ts_n, ko=kt)
    whi_v = w_hi[:, :].rearrange("p (ko e) -> p ko e", ko=kt)
    whi8_v = w_hi8[:, :].rearrange("p (ko e) -> p ko e", ko=kt)
    wlo8_v = w_lo8[:, :].rearrange("p (ko e) -> p ko e", ko=kt)

    with tile.TileContext(nc) as tc:
        with (
            tc.tile_pool(name="wpool", bufs=1) as wpool,
            tc.tile_pool(name="hpool", bufs=3) as hpool,
            tc.tile_pool(name="lpool", bufs=3) as lpool,
            tc.tile_pool(name="spool", bufs=4) as spool,
            tc.tile_pool(name="psum", bufs=4, space="PSUM") as psum_pool,
        ):
            # output staging: small per-tile results accumulate here and leave
            # as two large descriptor DMAs at the end (tiny per-tile DMAs get
            # the DIRECT2D encoding whose single wait slot walrus overflows)
            stage_idx = wpool.tile([P, ts_n, TOP_K], mybir.dt.int32, tag="sidx")
            stage_wv = wpool.tile([P, ts_n, TOP_K], f32, tag="swv")
            # gate weight: resident in SBUF for the whole kernel
            whi_t = wpool.tile([P, kt, e], fp16, tag="whi", name="whi")
            nc.sync.dma_start(whi_t, whi_v)
            whi8_t = wpool.tile([P, kt, e], fp8, tag="whi8", name="whi8")
            nc.sync.dma_start(whi8_t, whi8_v)
            wlo8_t = wpool.tile([P, kt, e], fp8, tag="wlo8", name="wlo8")
            nc.sync.dma_start(wlo8_t, wlo8_v)
            # DoubleRow [p, pair, plane, n] views of the resident fp8 weights
            whi8_pr = whi8_t[:, :, :].rearrange("p (kp two) e -> p kp two e", two=2)
            wlo8_pr = wlo8_t[:, :, :].rearrange("p (kp two) e -> p kp two e", two=2)
            for rep in range(repeat):
                for ts_i in range(ts_n):
                    hhi = hpool.tile(
                        [P, kt, P], fp16, tag="hhi", name=f"hhi{rep}_{ts_i}"
                    )
                    nc.sync.dma_start(hhi, hhi_v[:, ts_i])
                    hlo8 = hpool.tile(
                        [P, kt, P], fp8, tag="hlo8", name=f"hlo8{rep}_{ts_i}"
                    )
                    nc.sync.dma_start(hlo8, hlo8_v[:, ts_i])
                    # fp8 copy of the hi part: cast on the idle Pool engine
                    hhi8 = hpool.tile(
                        [P, kt, P], fp8, tag="hhi8", name=f"hhi8{rep}_{ts_i}"
                    )
                    nc.gpsimd.tensor_copy(hhi8, hhi)
                    hhi8_pr = hhi8[:, :, :].rearrange(
                        "p (kp two) t -> p kp two t", two=2
                    )
                    hlo8_pr = hlo8[:, :, :].rearrange(
                        "p (kp two) t -> p kp two t", two=2
                    )
                    pt = psum_pool.tile([P, 2 * e], f32, tag="pt")
                    # main: psum[:, 0:e] += hhi . whi   (fp16, 1 cyc/row)
                    for k in range(kt):
                        nc.tensor.matmul(
                            pt[:, :e],
                            hhi[:, k, :],
                            whi_t[:, k, :],
                            start=(k == 0),
                            stop=False,
                        )
                    # cross: psum[:, e:2e] += hhi.wlo + hlo.whi  (fp8 DR,
                    # 0.5 cyc/row; adjacent k-tiles ride the two DR planes)
                    for j in range(kp):
                        nc.tensor.matmul(
                            pt[:, e:],
                            hhi8_pr[:, j, :, :],
                            wlo8_pr[:, j, :, :],
                            start=False,
                            stop=False,
                            perf_mode=DR,
                        )
                    for j in range(kp):
                        nc.tensor.matmul(
                            pt[:, e:],
                            hlo8_pr[:, j, :, :],
                            whi8_pr[:, j, :, :],
                            start=False,
                            stop=(j == kp - 1),
                            perf_mode=DR,
                        )
                    # 64*logits = psum_hi + 2^-11 * psum_cross (order-preserving)
                    cross = lpool.tile([P, e], f32, tag="cross")
                    nc.vector.tensor_scalar_mul(cross, pt[:, e:], 1.0 / LO_SCALE)
                    m = lpool.tile([P, e], f32, tag="m")
                    nc.vector.tensor_add(m, pt[:, :e], cross)
                    mx = spool.tile([P, TOP_K], f32, tag="mx")
                    nc.vector.max(out=mx, in_=m)
                    idx_u = spool.tile([P, TOP_K], mybir.dt.uint32, tag="idxu")
                    nc.vector.max_index(idx_u, mx, m)
                    nc.vector.tensor_copy(stage_idx[:, ts_i, :], idx_u)
                    # normalized top-k softmax on true logits = raw/64:
                    # exp((raw - raw_max)/64) / sum
                    nm = spool.tile([P, 1], f32, tag="nm")
                    nc.vector.tensor_scalar_mul(nm, mx[:, 0:1], -1.0 / W_SCALE)
                    ev = spool.tile([P, TOP_K], f32, tag="ev")
                    sm = spool.tile([P, 1], f32, tag="sm")
                    nc.scalar.activation(
                        ev,
                        mx,
                        mybir.ActivationFunctionType.Exp,
                        bias=nm,
                        scale=1.0 / W_SCALE,
                        accum_out=sm,
                    )
                    rc = spool.tile([P, 1], f32, tag="rc")
                    nc.vector.reciprocal(rc, sm)
                    nc.vector.tensor_scalar_mul(stage_wv[:, ts_i, :], ev, rc)
            nc.sync.dma_start(
                idx_out[:, :].rearrange("(ts p) k -> p ts k", p=P), stage_idx
            )
            nc.sync.dma_start(
                w_out[:, :].rearrange("(ts p) k -> p ts k", p=P), stage_wv
            )
    nc.compile()
    return nc


def _get_nc():
    key = (T_LOC, HIDDEN, NUM_EXPERTS)
    if key not in _NC_CACHE:
        _NC_CACHE[key] = build_gate_nc(*key)
    return _NC_CACHE[key]


def _split_fp16(x, pre_scale=1.0):
    """x (f32) -> (hi, lo) fp16 with hi + lo/2^11 ~= pre_scale*x."""
    xs = x * np.float32(pre_scale) if pre_scale != 1.0 else x
    hi = xs.astype(np.float16)
    lo = ((xs - hi.astype(np.float32)) * np.float32(LO_SCALE)).astype(np.float16)
    return hi, lo


def _to_fp8(x):
    import ml_dtypes

    return np.asarray(x, np.float32).astype(ml_dtypes.float8_e4m3fn)


def _prep_inputs(hs, w):
    wT = np.ascontiguousarray(w.T)  # [H, E]
    w_hi, w_lo = _split_fp16(wT, W_SCALE)
    # [H, E] -> [p, ko, E] -> flat [P, KT*E]
    def wpack(x):
        return np.ascontiguousarray(
            x.reshape(KT, P, NUM_EXPERTS).transpose(1, 0, 2)
        ).reshape(P, KT * NUM_EXPERTS)

    w_hi_p = wpack(w_hi)
    w_hi8_p = _to_fp8(wpack(w_hi.astype(np.float32)))
    w_lo8_p = _to_fp8(wpack(w_lo.astype(np.float32)))
    in_maps = []
    for c in range(N_CORES):
        hsT_c = np.ascontiguousarray(hs[c * T_LOC : (c + 1) * T_LOC].T)  # [H,Tl]
        hs_hi, hs_lo = _split_fp16(hsT_c)
        # [H, Tl] = [ko*P, ts*P] -> [p, ts, ko, t]
        def hpack(x):
            return np.ascontiguousarray(
                x.reshape(KT, P, TS, P).transpose(1, 2, 0, 3)
            ).reshape(P, TS * KT * P)

        in_maps.append(
            {
                "hs_hi": hpack(hs_hi),
                "hs_lo8": _to_fp8(hpack(hs_lo.astype(np.float32))),
                "w_hi": w_hi_p,
                "w_hi8": w_hi8_p,
                "w_lo8": w_lo8_p,
            }
        )
    return in_maps


_FN_CACHE = {}


def _build_jit(nc, donate=True):
    """Build the reusable 8-core PJRT callable (same lowering path as
    run_bass_kernel_spmd under axon, but cached so repeat kernel() calls
    skip re-tracing/compiling)."""
    import jax
    import concourse.mybir as mybir
    from concourse import bass2jax
    from jax.sharding import Mesh, NamedSharding, PartitionSpec
    from jax.experimental.shard_map import shard_map

    bass2jax.install_neuronx_cc_hook()
    partition_name = nc.partition_id_tensor.name if nc.partition_id_tensor else None
    in_names, out_names, out_avals, zero_shapes = [], [], [], []
    for alloc in nc.m.functions[0].allocations:
        if not isinstance(alloc, mybir.MemoryLocationSet):
            continue
        name = alloc.memorylocations[0].name
        if alloc.kind == "ExternalInput":
            if name != partition_name:
                in_names.append(name)
        elif alloc.kind == "ExternalOutput":
            shape = tuple(alloc.tensor_shape)
            dtype = mybir.dt.np(alloc.dtype)
            out_names.append(name)
            out_avals.append(jax.core.ShapedArray(shape, dtype))
            zero_shapes.append((shape, dtype))
    n_params = len(in_names)
    n_outs = len(out_avals)
    all_in_names = list(in_names) + list(out_names)
    if partition_name is not None:
        all_in_names.append(partition_name)

    def _body(*args):
        operands = list(args)
        if partition_name is not None:
            operands.append(bass2jax.partition_id_tensor())
        outs = bass2jax._bass_exec_p.bind(
            *operands,
            out_avals=tuple(out_avals),
            in_names=tuple(all_in_names),
            out_names=tuple(out_names),
            lowering_input_output_aliases=(),
            sim_require_finite=True,
            sim_require_nnan=True,
            nc=nc,
        )
        return tuple(outs)

    devices = jax.devices()[:N_CORES]
    mesh = Mesh(np.asarray(devices), ("core",))
    in_specs = (PartitionSpec("core"),) * (n_params + n_outs)
    out_specs = (PartitionSpec("core"),) * len(out_names)
    donate_argnums = tuple(range(n_params, n_params + n_outs)) if donate else ()
    fn = jax.jit(
        shard_map(
            _body, mesh=mesh, in_specs=in_specs, out_specs=out_specs, check_rep=False
        ),
        donate_argnums=donate_argnums,
        keep_unused=True,
    )
    sharding = NamedSharding(mesh, PartitionSpec("core"))
    return fn, in_names, out_names, out_avals, zero_shapes, sharding


def _make_runner(nc):
    import jax

    fn, in_names, out_names, out_avals, zero_shapes, sharding = _build_jit(nc)

    def run(in_maps):
        concat_in = [
            np.concatenate(
                [np.asarray(in_maps[c][nm]) for c in range(N_CORES)], axis=0
            )
            for nm in in_names
        ]
        zeros = [np.zeros((N_CORES * s[0], *s[1:]), dt) for s, dt in zero_shapes]
        dev_in = [jax.device_put(x, sharding) for x in concat_in]
        out_arrs = fn(*dev_in, *zeros)
        return [
            {
                nm: np.asarray(out_arrs[i]).reshape(N_CORES, *out_avals[i].shape)[c]
                for i, nm in enumerate(out_names)
            }
            for c in range(N_CORES)
        ]

    return run


def kernel(hidden_states, weight):
    hs = np.asarray(hidden_states, dtype=np.float32)
    w = np.asarray(weight, dtype=np.float32)
    assert hs.shape == (NUM_TOKENS, HIDDEN), hs.shape
    assert w.shape == (NUM_EXPERTS, HIDDEN), w.shape

    in_maps = _prep_inputs(hs, w)
    nc = _get_nc()
    try:
        if "run" not in _FN_CACHE:
            _FN_CACHE["run"] = _make_runner(nc)
        results = _FN_CACHE["run"](in_maps)
    except Exception:
        # fall back to the stock path if the cached-runner path breaks
        from concourse.bass_utils import run_bass_kernel_spmd

        results = run_bass_kernel_spmd(
            nc, in_maps, core_ids=list(range(N_CORES))
        ).results

    topk_idx = np.concatenate([r["topk_idx"] for r in results], axis=0)
    topk_w = np.concatenate([r["topk_w"] for r in results], axis=0)
    row_idx = (
        np.arange(NUM_TOKENS * TOP_K, dtype=np.int32).reshape(TOP_K, NUM_TOKENS).T
    )
    return (
        topk_idx.astype(np.int32),
        topk_w.astype(np.float32),
        row_idx,
    )


# revision 8
# speedup vs baseline: 1.6601x; 1.6601x over previous
"""MoE gate (router) kernel for Trainium2.

Computes, for hidden_states [T, H] and gate weight [E, H]:
    logits = hidden_states @ weight.T          # [T, E]
    probs  = softmax(logits, axis=-1)
    topk_weight, topk_idx = top_k(probs, 8)    # normalized over the top-8
    row_idx = arange(T*8).reshape(8, T).T

Strategy (8 NeuronCores, data parallel over tokens):
  - Split hs = hhi + hlo/2^11 and 64*w = whi + wlo/2^11 (fp16 hi parts).
    logits*64 = hhi.whi  +  (hhi.wlo + hlo.whi)/2^11  (+ ~2^-22 dropped).
  - Main term hhi.whi in fp16 (1 cycle/row).  Both cross terms run in
    fp8e4 with the DoubleRow perf mode (0.5 cycles/row): adjacent k-tiles
    are paired as the two DoubleRow planes, so 28 DR matmuls cover 56
    k-tiles of hi.wlo and another 28 cover lo.whi.  Per k-tile the PE does
    256 (fp16) + 128 (fp8 DR) column-cycles = 384, vs 768 for an all-fp16
    hi/lo scheme and 1024 for native fp32.  The fp8 quantization only
    perturbs terms at 2^-11 scale: measured ~15/131072 top-8 index flips
    (idx rel err 7.6e-3, gate is 2e-2).
  - hs ships as one byte-packed DMA per 128-token tile: fp16 hi (2B) +
    fp8 lo (1B) = 3 B/elem, 64KB contiguous per partition; the fp8 copy
    of the hi part is cast on-device by DVE (4.5us/tile measured, vs
    24.5us on gpsimd), overlapped with PE work.  All DMAs use
    per-partition-contiguous runs so the DMA engines run at full width.
  - Top-8 via DVE max/max_index on the raw 64x-scaled logits (order
    invariant); the 1/64 folds into the exp activation's scale.  Softmax
    over 256 + top-k renorm reduces to softmax over the top-8 logits.
"""

import numpy as np

TOP_K = 8
NUM_EXPERTS = 256
HIDDEN = 7168
NUM_TOKENS = 16384
N_CORES = 8
T_LOC = NUM_TOKENS // N_CORES

W_SCALE = 64.0       # weight pre-scale so fp16(64*w) stays normal-range
LO_SCALE = 2048.0    # 2^11: lo parts carry the next 11 mantissa bits

P = 128
KT = HIDDEN // P     # 56 k-tiles along hidden dim
KP = KT // 2         # 28 DoubleRow k-pairs
TS = T_LOC // P      # 16 128-token subtiles per core

_NC_CACHE = {}


def build_gate_nc(t_loc=T_LOC, h=HIDDEN, e=NUM_EXPERTS, repeat=1):
    import concourse.mybir as mybir
    import concourse.tile as tile
    from concourse import bacc

    f32 = mybir.dt.float32
    fp16 = mybir.dt.float16
    fp8 = mybir.dt.float8e4
    DR = mybir.MatmulPerfMode.DoubleRow
    kt = h // P
    kp = kt // 2
    ts_n = t_loc // P

    u8 = mybir.dt.uint8
    nc = bacc.Bacc("TRN2", target_bir_lowering=False)
    # hs_raw[p, ts, :]: per (partition, token-tile) byte run = fp16 hi part
    # [ko, t] (2*kt*128 B) followed by fp8 lo part [ko, t] (kt*128 B), so each
    # 128-token tile arrives with ONE DMA of 3*kt*128 contiguous bytes per
    # partition.  hidden = ko*128+p, token = ts*128+t.
    hs_raw = nc.dram_tensor(
        "hs_raw", [P, ts_n * 3 * kt * P], u8, kind="ExternalInput"
    )
    # weights: whi fp16 + fp8 copies of whi and wlo (all [p, ko, e])
    w_hi = nc.dram_tensor("w_hi", [P, kt * e], fp16, kind="ExternalInput")
    w_hi8 = nc.dram_tensor("w_hi8", [P, kt * e], fp8, kind="ExternalInput")
    w_lo8 = nc.dram_tensor("w_lo8", [P, kt * e], fp8, kind="ExternalInput")
    idx_out = nc.dram_tensor(
        "topk_idx", [t_loc, TOP_K], mybir.dt.int32, kind="ExternalOutput"
    )
    w_out = nc.dram_tensor("topk_w", [t_loc, TOP_K], f32, kind="ExternalOutput")

    hraw_v = hs_raw[:, :].rearrange("p (ts b) -> p ts b", ts=ts_n)
    whi_v = w_hi[:, :].rearrange("p (ko e) -> p ko e", ko=kt)
    whi8_v = w_hi8[:, :].rearrange("p (ko e) -> p ko e", ko=kt)
    wlo8_v = w_lo8[:, :].rearrange("p (ko e) -> p ko e", ko=kt)

    with tile.TileContext(nc) as tc:
        with (
            tc.tile_pool(name="wpool", bufs=1) as wpool,
            tc.tile_pool(name="hpool", bufs=3) as hpool,
            tc.tile_pool(name="lpool", bufs=3) as lpool,
            tc.tile_pool(name="spool", bufs=4) as spool,
            tc.tile_pool(name="psum", bufs=4, space="PSUM") as psum_pool,
        ):
            # output staging: small per-tile results accumulate here and leave
            # as two large descriptor DMAs at the end (tiny per-tile DMAs get
            # the DIRECT2D encoding whose single wait slot walrus overflows)
            stage_idx = wpool.tile([P, ts_n, TOP_K], mybir.dt.int32, tag="sidx")
            stage_wv = wpool.tile([P, ts_n, TOP_K], f32, tag="swv")
            # gate weight: resident in SBUF for the whole kernel
            whi_t = wpool.tile([P, kt, e], fp16, tag="whi", name="whi")
            nc.sync.dma_start(whi_t, whi_v)
            whi8_t = wpool.tile([P, kt, e], fp8, tag="whi8", name="whi8")
            nc.sync.dma_start(whi8_t, whi8_v)
            wlo8_t = wpool.tile([P, kt, e], fp8, tag="wlo8", name="wlo8")
            nc.sync.dma_start(wlo8_t, wlo8_v)
            # DoubleRow [p, pair, plane, n] views of the resident fp8 weights
            whi8_pr = whi8_t[:, :, :].rearrange("p (kp two) e -> p kp two e", two=2)
            wlo8_pr = wlo8_t[:, :, :].rearrange("p (kp two) e -> p kp two e", two=2)
            for rep in range(repeat):
                for ts_i in range(ts_n):
                    hraw = hpool.tile(
                        [P, 3 * kt * P], u8, tag="hraw", name=f"hraw{rep}_{ts_i}"
                    )
                    nc.sync.dma_start(hraw, hraw_v[:, ts_i])
                    hhi = hraw[:, 0 : 2 * kt * P].bitcast(fp16).rearrange(
                        "p (ko t) -> p ko t", ko=kt
                    )
                    hlo8 = hraw[:, 2 * kt * P :].bitcast(fp8).rearrange(
                        "p (ko t) -> p ko t", ko=kt
                    )
                    # fp8 copy of the hi part, cast on DVE (gpsimd takes 24.5us
                    # per cast on HW vs DVE's 4.5us)
                    hhi8 = hpool.tile(
                        [P, kt, P], fp8, tag="hhi8", name=f"hhi8{rep}_{ts_i}"
                    )
                    nc.vector.tensor_copy(hhi8, hhi)
                    hhi8_pr = hhi8[:, :, :].rearrange(
                        "p (kp two) t -> p kp two t", two=2
                    )
                    hlo8_pr = hlo8[:, :, :].rearrange(
                        "p (kp two) t -> p kp two t", two=2
                    )
                    pt = psum_pool.tile([P, 2 * e], f32, tag="pt")
                    # main: psum[:, 0:e] += hhi . whi   (fp16, 1 cyc/row)
                    for k in range(kt):
                        nc.tensor.matmul(
                            pt[:, :e],
                            hhi[:, k, :],
                            whi_t[:, k, :],
                            start=(k == 0),
                            stop=False,
                        )
                    # cross: psum[:, e:2e] += hhi.wlo + hlo.whi  (fp8 DR,
                    # 0.5 cyc/row; adjacent k-tiles ride the two DR planes)
                    for j in range(kp):
                        nc.tensor.matmul(
                            pt[:, e:],
                            hhi8_pr[:, j, :, :],
                            wlo8_pr[:, j, :, :],
                            start=False,
                            stop=False,
                            perf_mode=DR,
                        )
                    for j in range(kp):
                        nc.tensor.matmul(
                            pt[:, e:],
                            hlo8_pr[:, j, :, :],
                            whi8_pr[:, j, :, :],
                            start=False,
                            stop=(j == kp - 1),
                            perf_mode=DR,
                        )
                    # 64*logits = psum_hi + 2^-11 * psum_cross (order-preserving)
                    cross = lpool.tile([P, e], f32, tag="cross")
                    nc.vector.tensor_scalar_mul(cross, pt[:, e:], 1.0 / LO_SCALE)
                    m = lpool.tile([P, e], f32, tag="m")
                    nc.vector.tensor_add(m, pt[:, :e], cross)
                    mx = spool.tile([P, TOP_K], f32, tag="mx")
                    nc.vector.max(out=mx, in_=m)
                    idx_u = spool.tile([P, TOP_K], mybir.dt.uint32, tag="idxu")
                    nc.vector.max_index(idx_u, mx, m)
                    nc.vector.tensor_copy(stage_idx[:, ts_i, :], idx_u)
                    # normalized top-k softmax on true logits = raw/64:
                    # exp((raw - raw_max)/64) / sum
                    nm = spool.tile([P, 1], f32, tag="nm")
                    nc.vector.tensor_scalar_mul(nm, mx[:, 0:1], -1.0 / W_SCALE)
                    ev = spool.tile([P, TOP_K], f32, tag="ev")
                    sm = spool.tile([P, 1], f32, tag="sm")
                    nc.scalar.activation(
                        ev,
                        mx,
                        mybir.ActivationFunctionType.Exp,
                        bias=nm,
                        scale=1.0 / W_SCALE,
                        accum_out=sm,
                    )
                    rc = spool.tile([P, 1], f32, tag="rc")
                    nc.vector.reciprocal(rc, sm)
                    nc.vector.tensor_scalar_mul(stage_wv[:, ts_i, :], ev, rc)
            nc.sync.dma_start(
                idx_out[:, :].rearrange("(ts p) k -> p ts k", p=P), stage_idx
            )
            nc.sync.dma_start(
                w_out[:, :].rearrange("(ts p) k -> p ts k", p=P), stage_wv
            )
    nc.compile()
    return nc


def _get_nc():
    key = (T_LOC, HIDDEN, NUM_EXPERTS)
    if key not in _NC_CACHE:
        _NC_CACHE[key] = build_gate_nc(*key)
    return _NC_CACHE[key]


def _split_fp16(x, pre_scale=1.0):
    """x (f32) -> (hi, lo) fp16 with hi + lo/2^11 ~= pre_scale*x."""
    xs = x * np.float32(pre_scale) if pre_scale != 1.0 else x
    hi = xs.astype(np.float16)
    lo = ((xs - hi.astype(np.float32)) * np.float32(LO_SCALE)).astype(np.float16)
    return hi, lo


def _to_fp8(x):
    import ml_dtypes

    return np.asarray(x, np.float32).astype(ml_dtypes.float8_e4m3fn)


def _prep_inputs(hs, w):
    wT = np.ascontiguousarray(w.T)  # [H, E]
    w_hi, w_lo = _split_fp16(wT, W_SCALE)
    # [H, E] -> [p, ko, E] -> flat [P, KT*E]
    def wpack(x):
        return np.ascontiguousarray(
            x.reshape(KT, P, NUM_EXPERTS).transpose(1, 0, 2)
        ).reshape(P, KT * NUM_EXPERTS)

    w_hi_p = wpack(w_hi)
    w_hi8_p = _to_fp8(wpack(w_hi.astype(np.float32)))
    w_lo8_p = _to_fp8(wpack(w_lo.astype(np.float32)))
    in_maps = []
    for c in range(N_CORES):
        hsT_c = np.ascontiguousarray(hs[c * T_LOC : (c + 1) * T_LOC].T)  # [H,Tl]
        hs_hi, hs_lo = _split_fp16(hsT_c)
        # [H, Tl] = [ko*P, ts*P] -> [p, ts, ko, t]
        def hpack(x):
            return np.ascontiguousarray(
                x.reshape(KT, P, TS, P).transpose(1, 2, 0, 3)
            ).reshape(P, TS * KT * P)

        hi_b = hpack(hs_hi).reshape(P, TS, KT * P).view(np.uint8)  # [P,TS,2*KT*P]
        lo_b = _to_fp8(hpack(hs_lo.astype(np.float32))).reshape(
            P, TS, KT * P
        ).view(np.uint8)  # [P, TS, KT*P]
        hs_raw = np.ascontiguousarray(
            np.concatenate([hi_b, lo_b], axis=2)
        ).reshape(P, TS * 3 * KT * P)
        in_maps.append(
            {
                "hs_raw": hs_raw,
                "w_hi": w_hi_p,
                "w_hi8": w_hi8_p,
                "w_lo8": w_lo8_p,
            }
        )
    return in_maps


_FN_CACHE = {}


def _build_jit(nc, donate=True):
    """Build the reusable 8-core PJRT callable (same lowering path as
    run_bass_kernel_spmd under axon, but cached so repeat kernel() calls
    skip re-tracing/compiling)."""
    import jax
    import concourse.mybir as mybir
    from concourse import bass2jax
    from jax.sharding import Mesh, NamedSharding, PartitionSpec
    from jax.experimental.shard_map import shard_map

    bass2jax.install_neuronx_cc_hook()
    partition_name = nc.partition_id_tensor.name if nc.partition_id_tensor else None
    in_names, out_names, out_avals, zero_shapes = [], [], [], []
    for alloc in nc.m.functions[0].allocations:
        if not isinstance(alloc, mybir.MemoryLocationSet):
            continue
        name = alloc.memorylocations[0].name
        if alloc.kind == "ExternalInput":
            if name != partition_name:
                in_names.append(name)
        elif alloc.kind == "ExternalOutput":
            shape = tuple(alloc.tensor_shape)
            dtype = mybir.dt.np(alloc.dtype)
            out_names.append(name)
            out_avals.append(jax.core.ShapedArray(shape, dtype))
            zero_shapes.append((shape, dtype))
    n_params = len(in_names)
    n_outs = len(out_avals)
    all_in_names = list(in_names) + list(out_names)
    if partition_name is not None:
        all_in_names.append(partition_name)

    def _body(*args):
        operands = list(args)
        if partition_name is not None:
            operands.append(bass2jax.partition_id_tensor())
        outs = bass2jax._bass_exec_p.bind(
            *operands,
            out_avals=tuple(out_avals),
            in_names=tuple(all_in_names),
            out_names=tuple(out_names),
            lowering_input_output_aliases=(),
            sim_require_finite=True,
            sim_require_nnan=True,
            nc=nc,
        )
        return tuple(outs)

    devices = jax.devices()[:N_CORES]
    mesh = Mesh(np.asarray(devices), ("core",))
    in_specs = (PartitionSpec("core"),) * (n_params + n_outs)
    out_specs = (PartitionSpec("core"),) * len(out_names)
    donate_argnums = tuple(range(n_params, n_params + n_outs)) if donate else ()
    fn = jax.jit(
        shard_map(
            _body, mesh=mesh, in_specs=in_specs, out_specs=out_specs, check_rep=False
        ),
        donate_argnums=donate_argnums,
        keep_unused=True,
    )
    sharding = NamedSharding(mesh, PartitionSpec("core"))
    return fn, in_names, out_names, out_avals, zero_shapes, sharding


def _make_runner(nc):
    import jax

    fn, in_names, out_names, out_avals, zero_shapes, sharding = _build_jit(nc)

    def run(in_maps):
        concat_in = [
            np.concatenate(
                [np.asarray(in_maps[c][nm]) for c in range(N_CORES)], axis=0
            )
            for nm in in_names
        ]
        zeros = [np.zeros((N_CORES * s[0], *s[1:]), dt) for s, dt in zero_shapes]
        dev_in = [jax.device_put(x, sharding) for x in concat_in]
        out_arrs = fn(*dev_in, *zeros)
        return [
            {
                nm: np.asarray(out_arrs[i]).reshape(N_CORES, *out_avals[i].shape)[c]
                for i, nm in enumerate(out_names)
            }
            for c in range(N_CORES)
        ]

    return run


def kernel(hidden_states, weight):
    hs = np.asarray(hidden_states, dtype=np.float32)
    w = np.asarray(weight, dtype=np.float32)
    assert hs.shape == (NUM_TOKENS, HIDDEN), hs.shape
    assert w.shape == (NUM_EXPERTS, HIDDEN), w.shape

    in_maps = _prep_inputs(hs, w)
    nc = _get_nc()
    try:
        if "run" not in _FN_CACHE:
            _FN_CACHE["run"] = _make_runner(nc)
        results = _FN_CACHE["run"](in_maps)
    except Exception:
        # fall back to the stock path if the cached-runner path breaks
        from concourse.bass_utils import run_bass_kernel_spmd

        results = run_bass_kernel_spmd(
            nc, in_maps, core_ids=list(range(N_CORES))
        ).results

    topk_idx = np.concatenate([r["topk_idx"] for r in results], axis=0)
    topk_w = np.concatenate([r["topk_w"] for r in results], axis=0)
    row_idx = (
        np.arange(NUM_TOKENS * TOP_K, dtype=np.int32).reshape(TOP_K, NUM_TOKENS).T
    )
    return (
        topk_idx.astype(np.int32),
        topk_w.astype(np.float32),
        row_idx,
    )


# revision 9
# speedup vs baseline: 1.6833x; 1.0140x over previous
"""MoE gate (router) kernel for Trainium2.

Computes, for hidden_states [T, H] and gate weight [E, H]:
    logits = hidden_states @ weight.T          # [T, E]
    probs  = softmax(logits, axis=-1)
    topk_weight, topk_idx = top_k(probs, 8)    # normalized over the top-8
    row_idx = arange(T*8).reshape(8, T).T

Strategy (8 NeuronCores, data parallel over tokens):
  - Split hs = hhi + hlo/2^11 and 64*w = whi + wlo/2^11 (fp16 hi parts).
    logits*64 = hhi.whi  +  (hhi.wlo + hlo.whi)/2^11  (+ ~2^-22 dropped).
  - Main term hhi.whi in fp16 (1 cycle/row).  Both cross terms run in
    fp8e4 with the DoubleRow perf mode (0.5 cycles/row): adjacent k-tiles
    are paired as the two DoubleRow planes, so 28 DR matmuls cover 56
    k-tiles of hi.wlo and another 28 cover lo.whi.  Per k-tile the PE does
    256 (fp16) + 128 (fp8 DR) column-cycles = 384, vs 768 for an all-fp16
    hi/lo scheme and 1024 for native fp32.  The fp8 quantization only
    perturbs terms at 2^-11 scale: measured ~15/131072 top-8 index flips
    (idx rel err 7.6e-3, gate is 2e-2).
  - hs ships as one byte-packed DMA per 128-token tile: fp16 hi (2B) +
    fp8 lo (1B) = 3 B/elem, 64KB contiguous per partition; the fp8 copy
    of the hi part is cast on-device by DVE (4.5us/tile measured, vs
    24.5us on gpsimd), overlapped with PE work.  All DMAs use
    per-partition-contiguous runs so the DMA engines run at full width.
  - Top-8 via DVE max/max_index on the raw 64x-scaled logits (order
    invariant); the 1/64 folds into the exp activation's scale.  Softmax
    over 256 + top-k renorm reduces to softmax over the top-8 logits.
"""

import numpy as np

TOP_K = 8
NUM_EXPERTS = 256
HIDDEN = 7168
NUM_TOKENS = 16384
N_CORES = 8
T_LOC = NUM_TOKENS // N_CORES

W_SCALE = 64.0       # weight pre-scale so fp16(64*w) stays normal-range
LO_SCALE = 2048.0    # 2^11: lo parts carry the next 11 mantissa bits

P = 128
KT = HIDDEN // P     # 56 k-tiles along hidden dim
KP = KT // 2         # 28 DoubleRow k-pairs
TS = T_LOC // P      # 16 128-token subtiles per core

_NC_CACHE = {}


def build_gate_nc(t_loc=T_LOC, h=HIDDEN, e=NUM_EXPERTS, repeat=1):
    import concourse.mybir as mybir
    import concourse.tile as tile
    from concourse import bacc

    f32 = mybir.dt.float32
    fp16 = mybir.dt.float16
    fp8 = mybir.dt.float8e4
    DR = mybir.MatmulPerfMode.DoubleRow
    kt = h // P
    kp = kt // 2
    ts_n = t_loc // P

    u8 = mybir.dt.uint8
    nc = bacc.Bacc("TRN2", target_bir_lowering=False)
    # hs_raw[p, ts, :]: per (partition, token-tile) byte run = fp16 hi part
    # [ko, t] (2*kt*128 B) followed by fp8 lo part [ko, t] (kt*128 B), so each
    # 128-token tile arrives with ONE DMA of 3*kt*128 contiguous bytes per
    # partition.  hidden = ko*128+p, token = ts*128+t.
    hs_raw = nc.dram_tensor(
        "hs_raw", [P, ts_n * 3 * kt * P], u8, kind="ExternalInput"
    )
    # weights: whi fp16 + fp8 copies of whi and wlo (all [p, ko, e])
    w_hi = nc.dram_tensor("w_hi", [P, kt * e], fp16, kind="ExternalInput")
    w_hi8 = nc.dram_tensor("w_hi8", [P, kt * e], fp8, kind="ExternalInput")
    w_lo8 = nc.dram_tensor("w_lo8", [P, kt * e], fp8, kind="ExternalInput")
    idx_out = nc.dram_tensor(
        "topk_idx", [t_loc, TOP_K], mybir.dt.int32, kind="ExternalOutput"
    )
    w_out = nc.dram_tensor("topk_w", [t_loc, TOP_K], f32, kind="ExternalOutput")

    hraw_v = hs_raw[:, :].rearrange("p (ts b) -> p ts b", ts=ts_n)
    whi_v = w_hi[:, :].rearrange("p (ko e) -> p ko e", ko=kt)
    whi8_v = w_hi8[:, :].rearrange("p (ko e) -> p ko e", ko=kt)
    wlo8_v = w_lo8[:, :].rearrange("p (ko e) -> p ko e", ko=kt)

    with tile.TileContext(nc) as tc:
        with (
            tc.tile_pool(name="wpool", bufs=1) as wpool,
            tc.tile_pool(name="hpool", bufs=4) as hpool,
            tc.tile_pool(name="lpool", bufs=3) as lpool,
            tc.tile_pool(name="spool", bufs=4) as spool,
            tc.tile_pool(name="psum", bufs=4, space="PSUM") as psum_pool,
        ):
            # output staging: small per-tile results accumulate here and leave
            # as two large descriptor DMAs at the end (tiny per-tile DMAs get
            # the DIRECT2D encoding whose single wait slot walrus overflows)
            stage_idx = wpool.tile([P, ts_n, TOP_K], mybir.dt.int32, tag="sidx")
            stage_wv = wpool.tile([P, ts_n, TOP_K], f32, tag="swv")
            # gate weight: resident in SBUF for the whole kernel
            whi_t = wpool.tile([P, kt, e], fp16, tag="whi", name="whi")
            nc.sync.dma_start(whi_t, whi_v)
            whi8_t = wpool.tile([P, kt, e], fp8, tag="whi8", name="whi8")
            nc.sync.dma_start(whi8_t, whi8_v)
            wlo8_t = wpool.tile([P, kt, e], fp8, tag="wlo8", name="wlo8")
            nc.sync.dma_start(wlo8_t, wlo8_v)
            # DoubleRow [p, pair, plane, n] views of the resident fp8 weights
            whi8_pr = whi8_t[:, :, :].rearrange("p (kp two) e -> p kp two e", two=2)
            wlo8_pr = wlo8_t[:, :, :].rearrange("p (kp two) e -> p kp two e", two=2)
            def load_tile(rep, ts_i):
                """DMA one 128-token tile and cast its fp8 hi copy (DVE).

                Emitted one tile AHEAD of the compute stream so the DVE
                cast lands before the previous tile's combine ops in
                program order -- the cast then overlaps the previous
                tile's matmuls instead of racing the DR stream's start.
                """
                hraw = hpool.tile(
                    [P, 3 * kt * P], u8, tag="hraw", name=f"hraw{rep}_{ts_i}"
                )
                nc.sync.dma_start(hraw, hraw_v[:, ts_i])
                hhi = hraw[:, 0 : 2 * kt * P].bitcast(fp16).rearrange(
                    "p (ko t) -> p ko t", ko=kt
                )
                hlo8 = hraw[:, 2 * kt * P :].bitcast(fp8).rearrange(
                    "p (ko t) -> p ko t", ko=kt
                )
                # fp8 copy of the hi part, cast on DVE (gpsimd takes 24.5us
                # per cast on HW vs DVE's 4.5us)
                hhi8 = hpool.tile(
                    [P, kt, P], fp8, tag="hhi8", name=f"hhi8{rep}_{ts_i}"
                )
                nc.vector.tensor_copy(hhi8, hhi)
                return hhi, hlo8, hhi8

            pending = None
            tiles = [
                (rep, ts_i) for rep in range(repeat) for ts_i in range(ts_n)
            ]
            for pos, (rep, ts_i) in enumerate(tiles):
                if pos == 0:
                    pending = load_tile(rep, ts_i)
                hhi, hlo8, hhi8 = pending
                if pos + 1 < len(tiles):
                    pending = load_tile(*tiles[pos + 1])
                if True:
                    hhi8_pr = hhi8[:, :, :].rearrange(
                        "p (kp two) t -> p kp two t", two=2
                    )
                    hlo8_pr = hlo8[:, :, :].rearrange(
                        "p (kp two) t -> p kp two t", two=2
                    )
                    pt = psum_pool.tile([P, 2 * e], f32, tag="pt")
                    # main: psum[:, 0:e] += hhi . whi   (fp16, 1 cyc/row)
                    for k in range(kt):
                        nc.tensor.matmul(
                            pt[:, :e],
                            hhi[:, k, :],
                            whi_t[:, k, :],
                            start=(k == 0),
                            stop=False,
                        )
                    # cross: psum[:, e:2e] += hhi.wlo + hlo.whi  (fp8 DR,
                    # 0.5 cyc/row; adjacent k-tiles ride the two DR planes)
                    for j in range(kp):
                        nc.tensor.matmul(
                            pt[:, e:],
                            hhi8_pr[:, j, :, :],
                            wlo8_pr[:, j, :, :],
                            start=False,
                            stop=False,
                            perf_mode=DR,
                        )
                    for j in range(kp):
                        nc.tensor.matmul(
                            pt[:, e:],
                            hlo8_pr[:, j, :, :],
                            whi8_pr[:, j, :, :],
                            start=False,
                            stop=(j == kp - 1),
                            perf_mode=DR,
                        )
                    # 64*logits = psum_hi + 2^-11 * psum_cross (order-preserving)
                    cross = lpool.tile([P, e], f32, tag="cross")
                    nc.vector.tensor_scalar_mul(cross, pt[:, e:], 1.0 / LO_SCALE)
                    m = lpool.tile([P, e], f32, tag="m")
                    nc.vector.tensor_add(m, pt[:, :e], cross)
                    mx = spool.tile([P, TOP_K], f32, tag="mx")
                    nc.vector.max(out=mx, in_=m)
                    idx_u = spool.tile([P, TOP_K], mybir.dt.uint32, tag="idxu")
                    nc.vector.max_index(idx_u, mx, m)
                    nc.vector.tensor_copy(stage_idx[:, ts_i, :], idx_u)
                    # normalized top-k softmax on true logits = raw/64:
                    # exp((raw - raw_max)/64) / sum
                    nm = spool.tile([P, 1], f32, tag="nm")
                    nc.vector.tensor_scalar_mul(nm, mx[:, 0:1], -1.0 / W_SCALE)
                    ev = spool.tile([P, TOP_K], f32, tag="ev")
                    sm = spool.tile([P, 1], f32, tag="sm")
                    nc.scalar.activation(
                        ev,
                        mx,
                        mybir.ActivationFunctionType.Exp,
                        bias=nm,
                        scale=1.0 / W_SCALE,
                        accum_out=sm,
                    )
                    rc = spool.tile([P, 1], f32, tag="rc")
                    nc.vector.reciprocal(rc, sm)
                    nc.vector.tensor_scalar_mul(stage_wv[:, ts_i, :], ev, rc)
            nc.sync.dma_start(
                idx_out[:, :].rearrange("(ts p) k -> p ts k", p=P), stage_idx
            )
            nc.sync.dma_start(
                w_out[:, :].rearrange("(ts p) k -> p ts k", p=P), stage_wv
            )
    nc.compile()
    return nc


def _get_nc():
    key = (T_LOC, HIDDEN, NUM_EXPERTS)
    if key not in _NC_CACHE:
        _NC_CACHE[key] = build_gate_nc(*key)
    return _NC_CACHE[key]


def _split_fp16(x, pre_scale=1.0):
    """x (f32) -> (hi, lo) fp16 with hi + lo/2^11 ~= pre_scale*x."""
    xs = x * np.float32(pre_scale) if pre_scale != 1.0 else x
    hi = xs.astype(np.float16)
    lo = ((xs - hi.astype(np.float32)) * np.float32(LO_SCALE)).astype(np.float16)
    return hi, lo


def _to_fp8(x):
    import ml_dtypes

    return np.asarray(x, np.float32).astype(ml_dtypes.float8_e4m3fn)


def _prep_inputs(hs, w):
    wT = np.ascontiguousarray(w.T)  # [H, E]
    w_hi, w_lo = _split_fp16(wT, W_SCALE)
    # [H, E] -> [p, ko, E] -> flat [P, KT*E]
    def wpack(x):
        return np.ascontiguousarray(
            x.reshape(KT, P, NUM_EXPERTS).transpose(1, 0, 2)
        ).reshape(P, KT * NUM_EXPERTS)

    w_hi_p = wpack(w_hi)
    w_hi8_p = _to_fp8(wpack(w_hi.astype(np.float32)))
    w_lo8_p = _to_fp8(wpack(w_lo.astype(np.float32)))
    in_maps = []
    for c in range(N_CORES):
        hsT_c = np.ascontiguousarray(hs[c * T_LOC : (c + 1) * T_LOC].T)  # [H,Tl]
        hs_hi, hs_lo = _split_fp16(hsT_c)
        # [H, Tl] = [ko*P, ts*P] -> [p, ts, ko, t]
        def hpack(x):
            return np.ascontiguousarray(
                x.reshape(KT, P, TS, P).transpose(1, 2, 0, 3)
            ).reshape(P, TS * KT * P)

        hi_b = hpack(hs_hi).reshape(P, TS, KT * P).view(np.uint8)  # [P,TS,2*KT*P]
        lo_b = _to_fp8(hpack(hs_lo.astype(np.float32))).reshape(
            P, TS, KT * P
        ).view(np.uint8)  # [P, TS, KT*P]
        hs_raw = np.ascontiguousarray(
            np.concatenate([hi_b, lo_b], axis=2)
        ).reshape(P, TS * 3 * KT * P)
        in_maps.append(
            {
                "hs_raw": hs_raw,
                "w_hi": w_hi_p,
                "w_hi8": w_hi8_p,
                "w_lo8": w_lo8_p,
            }
        )
    return in_maps


_FN_CACHE = {}


def _build_jit(nc, donate=True):
    """Build the reusable 8-core PJRT callable (same lowering path as
    run_bass_kernel_spmd under axon, but cached so repeat kernel() calls
    skip re-tracing/compiling)."""
    import jax
    import concourse.mybir as mybir
    from concourse import bass2jax
    from jax.sharding import Mesh, NamedSharding, PartitionSpec
    from jax.experimental.shard_map import shard_map

    bass2jax.install_neuronx_cc_hook()
    partition_name = nc.partition_id_tensor.name if nc.partition_id_tensor else None
    in_names, out_names, out_avals, zero_shapes = [], [], [], []
    for alloc in nc.m.functions[0].allocations:
        if not isinstance(alloc, mybir.MemoryLocationSet):
            continue
        name = alloc.memorylocations[0].name
        if alloc.kind == "ExternalInput":
            if name != partition_name:
                in_names.append(name)
        elif alloc.kind == "ExternalOutput":
            shape = tuple(alloc.tensor_shape)
            dtype = mybir.dt.np(alloc.dtype)
            out_names.append(name)
            out_avals.append(jax.core.ShapedArray(shape, dtype))
            zero_shapes.append((shape, dtype))
    n_params = len(in_names)
    n_outs = len(out_avals)
    all_in_names = list(in_names) + list(out_names)
    if partition_name is not None:
        all_in_names.append(partition_name)

    def _body(*args):
        operands = list(args)
        if partition_name is not None:
            operands.append(bass2jax.partition_id_tensor())
        outs = bass2jax._bass_exec_p.bind(
            *operands,
            out_avals=tuple(out_avals),
            in_names=tuple(all_in_names),
            out_names=tuple(out_names),
            lowering_input_output_aliases=(),
            sim_require_finite=True,
            sim_require_nnan=True,
            nc=nc,
        )
        return tuple(outs)

    devices = jax.devices()[:N_CORES]
    mesh = Mesh(np.asarray(devices), ("core",))
    in_specs = (PartitionSpec("core"),) * (n_params + n_outs)
    out_specs = (PartitionSpec("core"),) * len(out_names)
    donate_argnums = tuple(range(n_params, n_params + n_outs)) if donate else ()
    fn = jax.jit(
        shard_map(
            _body, mesh=mesh, in_specs=in_specs, out_specs=out_specs, check_rep=False
        ),
        donate_argnums=donate_argnums,
        keep_unused=True,
    )
    sharding = NamedSharding(mesh, PartitionSpec("core"))
    return fn, in_names, out_names, out_avals, zero_shapes, sharding


def _make_runner(nc):
    import jax

    fn, in_names, out_names, out_avals, zero_shapes, sharding = _build_jit(nc)

    def run(in_maps):
        concat_in = [
            np.concatenate(
                [np.asarray(in_maps[c][nm]) for c in range(N_CORES)], axis=0
            )
            for nm in in_names
        ]
        zeros = [np.zeros((N_CORES * s[0], *s[1:]), dt) for s, dt in zero_shapes]
        dev_in = [jax.device_put(x, sharding) for x in concat_in]
        out_arrs = fn(*dev_in, *zeros)
        return [
            {
                nm: np.asarray(out_arrs[i]).reshape(N_CORES, *out_avals[i].shape)[c]
                for i, nm in enumerate(out_names)
            }
            for c in range(N_CORES)
        ]

    return run


def kernel(hidden_states, weight):
    hs = np.asarray(hidden_states, dtype=np.float32)
    w = np.asarray(weight, dtype=np.float32)
    assert hs.shape == (NUM_TOKENS, HIDDEN), hs.shape
    assert w.shape == (NUM_EXPERTS, HIDDEN), w.shape

    in_maps = _prep_inputs(hs, w)
    nc = _get_nc()
    try:
        if "run" not in _FN_CACHE:
            _FN_CACHE["run"] = _make_runner(nc)
        results = _FN_CACHE["run"](in_maps)
    except Exception:
        # fall back to the stock path if the cached-runner path breaks
        from concourse.bass_utils import run_bass_kernel_spmd

        results = run_bass_kernel_spmd(
            nc, in_maps, core_ids=list(range(N_CORES))
        ).results

    topk_idx = np.concatenate([r["topk_idx"] for r in results], axis=0)
    topk_w = np.concatenate([r["topk_w"] for r in results], axis=0)
    row_idx = (
        np.arange(NUM_TOKENS * TOP_K, dtype=np.int32).reshape(TOP_K, NUM_TOKENS).T
    )
    return (
        topk_idx.astype(np.int32),
        topk_w.astype(np.float32),
        row_idx,
    )
